# revision 17
# baseline (speedup 1.0000x reference)
"""CrossAttentionBlock Trainium2 kernel (8 NeuronCores).

Sharding: 2-way data parallel over batch x 4-way tensor parallel
(attention: 3 of 12 heads per core; MLP: 512 of 2048 tokens per core
after a chunked ReduceScatter of the o-projection partials).

Key implementation points vs the straightforward version:
  - LN affine (w,b) folded into the projection weights/biases host-side;
    on-device LN is just bn_stats + a center-and-cast ACT pass; the 1/sigma
    scale is applied per-token in the projection epilogue (PSUM-read
    scalar_tensor_tensor).
  - Scores matmuls K=64 -> two kv-tiles run concurrently in the two
    64-row halves of the PE array (tile_position row packing, via
    duplicated K^T halves).
  - probs and V are fp8e4m3; attn^T x probs uses DoubleRow fp8 matmuls
    (256-deep contraction), with the softmax denominator as a 65th
    ones-column of V.  exp outputs fp8 with a x8 scale folded into the
    ACT bias (cancels in the normalization).
  - o-projection: K=64 second-half matmuls packed in pairs (duplicated
    wo rows), bias bo/4 added per core pre-ReduceScatter.
  - Attention runs chunk-major over 4 strided q-token chunks; each chunk
    finishes with its own bf16 ReduceScatter so the collective overlaps
    the next chunk's compute; MLP residual/cast prep runs per strip.
  - MLP in bf16 with ln_mlp folded into w1/b1, GELU bias on ACT,
    fused residual+bias epilogues.
Host: stitches out[b, 512s:512(s+1), :] = out_t.T
"""

import math

import numpy as np
import ml_dtypes

import concourse.bass as bass
import concourse.tile as tile
import concourse.mybir as mybir

BF = mybir.dt.bfloat16
F32 = mybir.dt.float32
I32 = mybir.dt.int32
F8 = mybir.dt.float8e4
AF = mybir.ActivationFunctionType
ALU = mybir.AluOpType
PM = mybir.MatmulPerfMode

DIM = 768
H = 12
DH = 64
MLP_H = 3072
B = 2
P = 2048          # both PQ and PKV
N_CORES = 8
G = 4             # cores per batch group
HPC = H // G      # heads per core = 3
TPC = P // G      # tokens per core for MLP = 512
ROPE_THETA = 100.0
LN_EPS = 1e-5
GROUPS = [[0, 1, 2, 3], [4, 5, 6, 7]]

NT = P // 128     # 16 token tiles
KT = DIM // 128   # 6 feature tiles
NC_CH = 4         # attention/RS chunks (strided strips of 128 per dest)
PSC = 8.0         # probs fp8 pre-scale (cancels in softmax denominator)


def _split_multi_waits(nc, max_waits: int = 1):
    """Walrus codegen in this container accepts at most one sync wait per
    instruction; Tile's post-scheduler drain/barrier can carry more. Move
    the excess onto same-engine nops inserted just before."""
    for bb in nc.main_func.blocks:
        i = 0
        insts = bb.instructions
        while i < len(insts):
            ins = insts[i]
            si = ins.sync_info
            if si is not None and si.on_wait and len(si.on_wait) > max_waits:
                waits = list(si.on_wait)
                keep = waits[-max_waits:]
                extra = waits[:-max_waits]
                nops = []
                for w in extra:
                    nop = mybir.InstNoOp(
                        name=f"I-waitfix-{nc.next_id()}", engine=ins.engine
                    )
                    nop.sync_info = mybir.SyncInfo(on_wait=[w], on_update=[])
                    nops.append(nop)
                ins.sync_info = mybir.SyncInfo(
                    on_wait=keep, on_update=list(si.on_update or [])
                )
                for j, nop in enumerate(nops):
                    insts.insert(i + j, nop)
                i += len(nops)
            i += 1


def build_nc():
    nc = bass.Bass("TRN2", target_bir_lowering=False, debug=False,
                   num_devices=N_CORES)

    # ---------------- inputs ----------------
    query = nc.dram_tensor("query", [P, DIM], F32, kind="ExternalInput")
    kv = nc.dram_tensor("kv", [P, DIM], F32, kind="ExternalInput")
    q_res_t = nc.dram_tensor("q_res_t", [DIM, TPC], F32, kind="ExternalInput")
    poscq = nc.dram_tensor("poscq", [128, NT], I32, kind="ExternalInput")
    posckv = nc.dram_tensor("posckv", [128, NT], I32, kind="ExternalInput")
    # combined-position trig rows: [cos_y(16)|cos_x(16)|sin_y(16)|sin_x(16)]
    tabcs = nc.dram_tensor("tabcs", [64 * 64, 64], BF, kind="ExternalInput")
    wq_s = nc.dram_tensor("wq_s", [DIM, HPC * DH], BF, kind="ExternalInput")
    wkv_s = nc.dram_tensor("wkv_s", [DIM, 2 * HPC * DH], BF, kind="ExternalInput")
    bqkv_s = nc.dram_tensor("bqkv_s", [3 * HPC * DH], F32, kind="ExternalInput")
    wo01_d = nc.dram_tensor("wo01_d", [128, DIM], BF, kind="ExternalInput")
    wo2d_d = nc.dram_tensor("wo2d_d", [128, DIM], BF, kind="ExternalInput")
    bo4 = nc.dram_tensor("bo4", [DIM], F32, kind="ExternalInput")
    w1 = nc.dram_tensor("w1", [128, KT // 2, 2, MLP_H], F8,
                        kind="ExternalInput")
    b1 = nc.dram_tensor("b1", [MLP_H], F32, kind="ExternalInput")
    w2 = nc.dram_tensor("w2", [128, MLP_H // 256, 2, DIM], F8,
                        kind="ExternalInput")
    b2 = nc.dram_tensor("b2", [DIM], F32, kind="ExternalInput")
    out_t = nc.dram_tensor("out_t", [DIM, TPC], F32, kind="ExternalOutput")

    def bcast_ap(t, n_part, free):
        return bass.AP(tensor=t.ap().tensor, offset=0,
                       ap=[[0, n_part], [1, free]])

    with tile.TileContext(nc) as tc:
        with (
            tc.tile_pool(name="consts", bufs=1) as consts,
            tc.tile_pool(name="work", bufs=3) as work,
            tc.tile_pool(name="dram", bufs=1, space="DRAM") as dram,
        ):
            # ---------------- constants ----------------
            ones_bf = consts.tile([128, 128], BF)
            nc.vector.memset(ones_bf[:], 1.0)
            eps_sb = consts.tile([128, 1], F32)
            nc.vector.memset(eps_sb[:], LN_EPS)
            ln8_sb = consts.tile([128, 1], F32)
            nc.vector.memset(ln8_sb[:], math.log(PSC))
            bo4_sb = consts.tile([128, KT], F32)
            nc.sync.dma_start(bo4_sb[:], bo4.rearrange("(m p) -> p m", p=128))
            b1_sb = consts.tile([128, MLP_H // 128], F32)
            nc.sync.dma_start(b1_sb[:], b1.rearrange("(m p) -> p m", p=128))
            b2_sb = consts.tile([128, KT], F32)
            nc.sync.dma_start(b2_sb[:], b2.rearrange("(m p) -> p m", p=128))
            poscq_sb = consts.tile([128, NT], I32)
            nc.sync.dma_start(poscq_sb[:], poscq[:])
            posckv_sb = consts.tile([128, NT], I32)
            nc.sync.dma_start(posckv_sb[:], posckv[:])
            bqkv_rep = consts.tile([128, 3 * HPC * DH], F32)
            nc.gpsimd.dma_start(bqkv_rep[:],
                                bcast_ap(bqkv_s, 128, 3 * HPC * DH))

            # DRAM staging
            xz_dram = dram.tile([2, P, DIM], BF)           # centered bf16 q/kv
            qrot_dram = dram.tile([P, 4 * DH], BF)   # [h0|h1|h2|h0]
            krot_dram = dram.tile([P, 4 * DH], BF)
            cc_in = []
            cc_out = []
            for c in range(NC_CH):
                cci = dram.tile([G, DIM, 128], BF, name=f"cc_in_{c}")
                cco = dram.tile([DIM, 128], BF, name=f"cc_out_{c}")
                cc_in.append(cci)
                cc_out.append(cco)

            # ---------- attention-lifetime tiles (outer pool) ----------
            with tc.tile_pool(name="attnP", bufs=1) as attnP:
                wo01_sb = attnP.tile([128, DIM], BF)
                nc.sync.dma_start(wo01_sb[:], wo01_d[:])
                wo2d_sb = attnP.tile([128, DIM], BF)
                nc.sync.dma_start(wo2d_sb[:], wo2d_d[:])
                v8 = attnP.tile([128, NT // 2, 2, HPC, 80], F8)
                nc.vector.memset(v8[:, :, :, :, DH:DH + 1], 1.0)
                qTd = attnP.tile([128, HPC, P], BF)
                kTd = attnP.tile([128, HPC, P], BF)

                # ------------ phase 1: LN + proj + RoPE ------------
                with (
                    tc.tile_pool(name="earlyP", bufs=1) as earlyP,
                    tc.tile_pool(name="lnwk", bufs=3) as lnwk,
                    tc.tile_pool(name="lnsc", bufs=2) as lnsc,
                    tc.tile_pool(name="ropeP", bufs=2) as ropeP,
                    tc.tile_pool(name="psProj", bufs=2, space="PSUM") as psProj,
                ):
                    def ln_side(side, src, r_all, mv_all):
                        # token-major LN stats + center-and-cast; bf16 to DRAM
                        for t in range(NT):
                            xt = lnwk.tile([128, DIM], F32, tag=f"lnx{side}")
                            nc.sync.dma_start(
                                xt[:], src[t * 128:(t + 1) * 128, :])
                            st = lnwk.tile([128, 2, nc.vector.BN_STATS_DIM],
                                           F32, tag=f"bnst{side}")
                            xg = xt[:].rearrange("p (g d) -> p g d", g=2)
                            for g in range(2):
                                nc.vector.bn_stats(st[:, g, :], xg[:, g, :])
                            nc.vector.bn_aggr(mv_all[:, t, :], st[:])
                            nm = lnsc.tile([128, 1], F32, tag=f"lnnm{side}")
                            nc.vector.tensor_scalar_mul(
                                nm[:], mv_all[:, t, 0:1], -1.0)
                            xzb = lnwk.tile([128, DIM], BF, tag=f"lnz{side}")
                            nc.scalar.activation(xzb[:], xt[:], AF.Identity,
                                                 bias=nm[:], scale=1.0)
                            nc.scalar.dma_start(
                                xz_dram[side, t * 128:(t + 1) * 128, :],
                                xzb[:])
                        # batched 1/sigma for all tiles
                        nc.scalar.activation(r_all[:], mv_all[:, :, 1],
                                             AF.Sqrt, bias=eps_sb[:],
                                             scale=1.0)
                        nc.vector.reciprocal(r_all[:], r_all[:])

                    def rope4(b4, src4, gt4, dst4):
                        # src4/dst4: [128, 4, HPC*64] fp32/bf16,
                        # layout per head: [a(2), j(16), two(2)]
                        sv = src4[:].rearrange(
                            "p t (h a j two) -> p t h a j two",
                            h=HPC, a=2, j=16, two=2)
                        dv = dst4[:].rearrange(
                            "p t (h a j two) -> p t h a j two",
                            h=HPC, a=2, j=16, two=2)
                        gv = gt4[:].rearrange("p t (cs a j) -> p t cs a j",
                                              cs=2, a=2)
                        cw = gv[:, :, None, 0, :, :].to_broadcast(
                            (128, 4, HPC, 2, 16))
                        sw = gv[:, :, None, 1, :, :].to_broadcast(
                            (128, 4, HPC, 2, 16))
                        xe = sv[:, :, :, :, :, 0]
                        xo = sv[:, :, :, :, :, 1]
                        t1 = ropeP.tile([128, 4, HPC, 2, 16], F32, tag="rt1")
                        t2 = ropeP.tile([128, 4, HPC, 2, 16], F32, tag="rt2")
                        ye = ropeP.tile([128, 4, HPC, 2, 16], F32, tag="rye")
                        nc.vector.tensor_tensor(t1[:], xo, sw, ALU.mult)
                        nc.vector.tensor_tensor(ye[:], xe, cw, ALU.mult)
                        nc.vector.tensor_tensor(dv[:, :, :, :, :, 0],
                                                ye[:], t1[:], ALU.subtract)
                        nc.vector.tensor_tensor(t2[:], xe, sw, ALU.mult)
                        nc.vector.tensor_tensor(ye[:], xo, cw, ALU.mult)
                        nc.vector.tensor_tensor(dv[:, :, :, :, :, 1],
                                                ye[:], t2[:], ALU.add)

                    def proj_side(side, nT_sb, r_all, pos_sb, is_q):
                        gt_all = earlyP.tile([128, NT, 64], BF,
                                             tag=f"gt{side}", name=f"gt{side}")
                        for t in range(NT):
                            nc.gpsimd.indirect_dma_start(
                                out=gt_all[:, t, :], out_offset=None,
                                in_=tabcs[:],
                                in_offset=bass.IndirectOffsetOnAxis(
                                    ap=pos_sb[:, t:t + 1], axis=0))
                        nout = HPC * DH if is_q else 2 * HPC * DH
                        w_sb = earlyP.tile([128, KT, nout], BF,
                                           tag=f"w{side}", name=f"w{side}")
                        nc.sync.dma_start(
                            w_sb[:],
                            (wq_s if is_q else wkv_s).rearrange(
                                "(k p) n -> p k n", p=128))
                        for b4 in range(NT // 4):
                            x4 = ropeP.tile([128, 4, HPC * DH], F32,
                                            tag=f"x4{side}")
                            for tt in range(4):
                                t = b4 * 4 + tt
                                ps = psProj.tile([128, nout], F32,
                                                 tag=f"ps{side}")
                                for k in range(KT):
                                    nc.tensor.matmul(
                                        ps[:],
                                        nT_sb[:, k, t * 128:(t + 1) * 128],
                                        w_sb[:, k, :], start=(k == 0),
                                        stop=(k == KT - 1))
                                if is_q:
                                    nc.vector.scalar_tensor_tensor(
                                        x4[:, tt, :], ps[:],
                                        r_all[:, t:t + 1],
                                        bqkv_rep[:, 0:HPC * DH],
                                        ALU.mult, ALU.add)
                                else:
                                    nc.vector.scalar_tensor_tensor(
                                        x4[:, tt, :], ps[:, 0:HPC * DH],
                                        r_all[:, t:t + 1],
                                        bqkv_rep[:, HPC * DH:2 * HPC * DH],
                                        ALU.mult, ALU.add)
                                    nc.vector.scalar_tensor_tensor(
                                        v8[:, t // 2, t % 2, :, 0:DH],
                                        ps[:, HPC * DH:2 * HPC * DH].rearrange(
                                            "p (h d) -> p h d", h=HPC),
                                        r_all[:, t:t + 1],
                                        bqkv_rep[:, 2 * HPC * DH:
                                                 3 * HPC * DH].rearrange(
                                            "p (h d) -> p h d", h=HPC),
                                        ALU.mult, ALU.add)
                            r4 = ropeP.tile([128, 4, HPC * DH], BF,
                                            tag=f"r4{side}")
                            rope4(b4, x4, gt_all[:, b4 * 4:b4 * 4 + 4, :], r4)
                            rot = qrot_dram if is_q else krot_dram
                            nc.scalar.dma_start(
                                rot[b4 * 512:(b4 + 1) * 512,
                                    0:HPC * DH].rearrange(
                                    "(t p) f -> p t f", p=128),
                                r4[:])
                            nc.scalar.dma_start(
                                rot[b4 * 512:(b4 + 1) * 512,
                                    HPC * DH:4 * DH].rearrange(
                                    "(t p) f -> p t f", p=128),
                                r4[:, :, 0:DH])

                    # ---- kv side first (feeds deepest chain) ----
                    r_kv = earlyP.tile([128, NT], F32, name="r_kv")
                    mv_kv = earlyP.tile([128, NT, 2], F32, name="mv_kv")
                    ln_side(1, kv, r_kv, mv_kv)
                    nT_kv = earlyP.tile([128, KT, P], BF, tag="nT",
                                        name="nTkv")
                    for half in range(2):
                        hsl = slice(half * 1024, (half + 1) * 1024)
                        for m in range(KT):
                            nc.scalar.dma_start_transpose(
                                nT_kv[:, m, hsl],
                                xz_dram[1, hsl, m * 128:(m + 1) * 128])
                    proj_side(1, nT_kv, r_kv, posckv_sb, False)
                    # slots: 0=[h0|h1], 1=[h1|h2], 2=[h2|h0]
                    for s in range(HPC):
                        nc.sync.dma_start_transpose(
                            kTd[:, s, :],
                            krot_dram[:, s * 64:s * 64 + 128])

                    # ---- q side ----
                    r_q = earlyP.tile([128, NT], F32, name="r_q")
                    mv_q = earlyP.tile([128, NT, 2], F32, name="mv_q")
                    ln_side(0, query, r_q, mv_q)
                    nT_q = earlyP.tile([128, KT, P], BF, tag="nT", name="nTq")
                    for half in range(2):
                        hsl = slice(half * 1024, (half + 1) * 1024)
                        for m in range(KT):
                            nc.scalar.dma_start_transpose(
                                nT_q[:, m, hsl],
                                xz_dram[0, hsl, m * 128:(m + 1) * 128])
                    proj_side(0, nT_q, r_q, poscq_sb, True)
                    for s in range(HPC):
                        nc.sync.dma_start_transpose(
                            qTd[:, s, :],
                            qrot_dram[:, s * 64:s * 64 + 128])

                # ---------- MLP weights: DMA during attention ----------
                with (
                    tc.tile_pool(name="mlpw", bufs=1) as mlpw,
                    tc.tile_pool(name="mlpP", bufs=1) as mlpP,
                ):
                    w1_sb = mlpw.tile([128, KT // 2, 2, MLP_H], F8)
                    nc.gpsimd.dma_start(w1_sb[:], w1[:])
                    w2_sb = mlpw.tile([128, MLP_H // 256, 2, DIM], F8)
                    nc.gpsimd.dma_start(w2_sb[:], w2[:])
                    qres_sb = mlpP.tile([128, KT, TPC], F32)
                    nc.gpsimd.dma_start(
                        qres_sb[:],
                        q_res_t[:].rearrange("(m p) n -> p m n", p=128))
                    x_sb = mlpP.tile([128, KT, TPC], F32)
                    xb_sb = mlpP.tile([128, KT, TPC], BF)
                    sqb_sb = mlpP.tile([128, KT, TPC], BF)

                    # ------------ phase 2: attention, chunk-major ------------
                    attn_ctx = (
                        tc.tile_pool(name="attwk", bufs=2),
                        tc.tile_pool(name="prbP", bufs=3),
                        tc.tile_pool(name="psS", bufs=2, space="PSUM"),
                        tc.tile_pool(name="psA", bufs=2, space="PSUM"),
                        tc.tile_pool(name="psM", bufs=2, space="PSUM"),
                    )
                    attwk = attn_ctx[0].__enter__()
                    prbP = attn_ctx[1].__enter__()
                    psS = attn_ctx[2].__enter__()
                    psA = attn_ctx[3].__enter__()
                    psM = attn_ctx[4].__enter__()

                    def qchunk(tl, base, h, c):
                        # strided 4x128 q-token chunk view [64, 4, 128]
                        return tl[base:base + 64, h, :].rearrange(
                            "p (s j) -> p s j", s=4)[:, :, c * 128:(c + 1) * 128]

                    for c in range(NC_CH):
                        at01c = attwk.tile([128, 4, 128], BF, tag="at01")
                        at2dc = attwk.tile([128, 4, 128], BF, tag="at2d")
                        for h in range(HPC):
                            att_ps = psA.tile([65, 4, 128], F32, tag="att")
                            for tp in range(NT // 2):
                                s_ps = psS.tile([128, 2, 4, 128], F32,
                                                tag="s")
                                for ko in range(2):
                                    i = 2 * tp + ko
                                    slot = h if ko == 0 else (h + 2) % 3
                                    nc.tensor.matmul(
                                        s_ps[:, ko, :, :],
                                        kTd[ko * 64:ko * 64 + 64, slot,
                                            i * 128:(i + 1) * 128],
                                        qchunk(qTd, ko * 64, slot, c),
                                        start=True, stop=True)
                                prb = prbP.tile([128, 2, 4, 128], F8,
                                                tag="prb")
                                nc.scalar.activation(
                                    prb[:].rearrange("p a s j -> p (a s j)"),
                                    s_ps[:].rearrange("p a s j -> p (a s j)"),
                                    AF.Exp, bias=ln8_sb[:], scale=DH ** -0.5)
                                nc.tensor.matmul(
                                    att_ps[:].rearrange("p s j -> p (s j)"),
                                    v8[:, tp, :, h, 0:DH + 1],
                                    prb[:].rearrange("p a s j -> p a (s j)"),
                                    start=(tp == 0), stop=(tp == NT // 2 - 1),
                                    perf_mode=PM.DoubleRow)
                            # normalize
                            att_bf = attwk.tile([65, 4, 128], BF, tag="attbf")
                            nc.vector.tensor_copy(att_bf[:], att_ps[:])
                            with nc.allow_low_precision(
                                    reason="softmax denom recip in bf16"):
                                nc.vector.reciprocal(
                                    att_bf[64:65, :, :], att_bf[64:65, :, :])
                            rep_t = psM.tile([128, 4, 128], F32, tag="m",
                                 name=f"rep_{c}_{h}")
                            rep_ps = rep_t[0:64]
                            nc.tensor.matmul(
                                rep_ps[:].rearrange("p s j -> p (s j)"),
                                ones_bf[64:65, 0:64],
                                att_bf[64:65, :, :].rearrange(
                                    "p s j -> p (s j)"),
                                start=True, stop=True)
                            if h == 0:
                                nc.vector.tensor_tensor(
                                    at01c[0:64, :, :], att_bf[0:64, :, :],
                                    rep_ps[:], ALU.mult)
                            elif h == 1:
                                h1st = attwk.tile([64, 4, 128], BF,
                                                  tag="h1st")
                                nc.vector.tensor_tensor(
                                    h1st[:], att_bf[0:64, :, :], rep_ps[:],
                                    ALU.mult)
                                nc.scalar.dma_start(at01c[64:128, :, :],
                                                    h1st[:])
                            else:
                                nc.vector.tensor_tensor(
                                    at2dc[0:64, :, :], att_bf[0:64, :, :],
                                    rep_ps[:], ALU.mult)
                                nc.scalar.dma_start(at2dc[64:128, :, :],
                                                    at2dc[0:64, :, :])

                        # ---- o-proj for this chunk ----
                        at01f = at01c[:].rearrange("p s j -> p (s j)")
                        at2f0 = at2dc[0:64, :, :].rearrange("p s j -> p (s j)")
                        at2f1 = at2dc[64:128, :, :].rearrange(
                            "p s j -> p (s j)")
                        xo_all = attwk.tile([128, 4, KT, 128], BF,
                                            tag="xoall", name=f"xo_all{c}")
                        for mp in range(KT // 2):
                            xo_ps = [psM.tile([128, 4, 128], F32,
                                              tag="m",
                                              name=f"xo_{c}_{mp}_{mm}")
                                     for mm in range(2)]
                            for mm in range(2):
                                m = mp * 2 + mm
                                nc.tensor.matmul(
                                    xo_ps[mm][:].rearrange("p s j -> p (s j)"),
                                    wo01_sb[:, m * 128:(m + 1) * 128],
                                    at01f, start=True, stop=False)
                            nc.tensor.matmul(
                                xo_ps[0][:].rearrange("p s j -> p (s j)"),
                                wo2d_sb[0:64, (mp * 2) * 128:
                                        (mp * 2 + 1) * 128],
                                at2f0, start=False, stop=True)
                            nc.tensor.matmul(
                                xo_ps[1][:].rearrange("p s j -> p (s j)"),
                                wo2d_sb[64:128, (mp * 2 + 1) * 128:
                                        (mp * 2 + 2) * 128],
                                at2f1, start=False, stop=True)
                            for mm in range(2):
                                m = mp * 2 + mm
                                nc.vector.tensor_scalar(
                                    xo_all[:, :, m, :], xo_ps[mm][:],
                                    bo4_sb[:, m:m + 1], None, ALU.add,
                                    ALU.bypass)
                        nc.sync.dma_start(
                            cc_in[c][:].rearrange("s (m p) j -> p (s m) j",
                                                  p=128),
                            xo_all[:].rearrange("p s m j -> p (s m) j"))

                        nc.gpsimd.collective_compute(
                            "ReduceScatter", ALU.add, replica_groups=GROUPS,
                            ins=[cc_in[c][:].opt()],
                            outs=[cc_out[c][:].opt()])

                        # ---- strip prep: x = cc + qres; bf16 + square ----
                        csl = slice(c * 128, (c + 1) * 128)
                        xin = work.tile([128, KT, 128], BF, tag="xin")
                        nc.sync.dma_start(
                            xin[:],
                            cc_out[c][:].rearrange("(m p) n -> p m n",
                                                   p=128))
                        nc.vector.tensor_tensor(
                            x_sb[:, :, csl], xin[:], qres_sb[:, :, csl],
                            ALU.add)
                        nc.vector.tensor_copy(xb_sb[:, :, csl],
                                              x_sb[:, :, csl])
                        nc.vector.tensor_tensor(
                            sqb_sb[:, :, csl], xb_sb[:, :, csl],
                            xb_sb[:, :, csl], ALU.mult)

                    for _p in reversed(attn_ctx):
                        _p.__exit__(None, None, None)

                    # ------------ phase 3: MLP ------------
                    xn_sb = mlpP.tile([128, KT, TPC], F8)
                    with tc.tile_pool(name="psStat", bufs=1,
                                      space="PSUM") as psStat:
                        mean_ps = psStat.tile([1, TPC], F32, tag="meanps")
                        sq_ps = psStat.tile([1, TPC], F32, tag="sqps")
                        for m in range(KT):
                            nc.tensor.matmul(mean_ps[:], ones_bf[:, 0:1],
                                             xb_sb[:, m, :], start=(m == 0),
                                             stop=(m == KT - 1))
                        for m in range(KT):
                            nc.tensor.matmul(sq_ps[:], ones_bf[:, 0:1],
                                             sqb_sb[:, m, :], start=(m == 0),
                                             stop=(m == KT - 1))
                        mrow_bf = mlpP.tile([1, TPC], BF)
                        rrow_bf = mlpP.tile([1, TPC], BF)
                        mrow = work.tile([1, TPC], F32, tag="mrow")
                        vrow = work.tile([1, TPC], F32, tag="vrow")
                        nc.vector.tensor_scalar_mul(mrow[:], mean_ps[:],
                                                    1.0 / DIM)
                        nc.vector.tensor_scalar_mul(vrow[:], sq_ps[:],
                                                    1.0 / DIM)
                        msq = work.tile([1, TPC], F32, tag="msq")
                        nc.vector.tensor_mul(msq[:], mrow[:], mrow[:])
                        nc.vector.tensor_tensor(vrow[:], vrow[:], msq[:],
                                                ALU.subtract)
                        nc.scalar.activation(vrow[:], vrow[:], AF.Sqrt,
                                             bias=eps_sb[0:1, :], scale=1.0)
                        nc.vector.reciprocal(vrow[:], vrow[:])
                        nc.vector.tensor_copy(rrow_bf[:], vrow[:])
                        nc.vector.tensor_copy(mrow_bf[:], mrow[:])
                    with tc.tile_pool(name="psReps", bufs=1,
                                      space="PSUM") as psReps:
                        mrep_ps = psReps.tile([128, TPC], F32, tag="mrep")
                        nc.tensor.matmul(mrep_ps[:], ones_bf[0:1, :],
                                         mrow_bf[:], start=True, stop=True)
                        rrep_ps = psReps.tile([128, TPC], F32, tag="rrep")
                        nc.tensor.matmul(rrep_ps[:], ones_bf[0:1, :],
                                         rrow_bf[:], start=True, stop=True)
                        for m in range(KT):
                            t1 = work.tile([128, TPC], F32, tag="mlnt1")
                            nc.vector.tensor_tensor(
                                t1[:], x_sb[:, m, :], mrep_ps[:],
                                ALU.subtract)
                            nc.vector.tensor_tensor(xn_sb[:, m, :], t1[:],
                                                    rrep_ps[:], ALU.mult)

                    # FF1 + GELU (b1 on ACT bias), FF2 streamed per jp
                    with (
                        tc.tile_pool(name="hP", bufs=3) as hP,
                        tc.tile_pool(name="psF1", bufs=1,
                                     space="PSUM") as psF1,
                        tc.tile_pool(name="psF2", bufs=1,
                                     space="PSUM") as psF2,
                    ):
                        f2 = [psF2.tile([128, TPC], F32, tag=f"f2_{m}",
                                        name=f"f2t_{m}")
                              for m in range(KT)]
                        NJP = MLP_H // 256
                        xn4 = xn_sb[:].rearrange("p (kp ko) n -> p kp ko n",
                                                 ko=2)
                        for jp in range(NJP):
                            f1 = psF1.tile([128, 2, TPC], F32, tag="f1")
                            hj = hP.tile([128, 2, TPC], F8, tag="h")
                            for jj in range(2):
                                j = jp * 2 + jj
                                for kp in range(KT // 2):
                                    nc.tensor.matmul(
                                        f1[:, jj, :],
                                        w1_sb[:, kp, :,
                                              j * 128:(j + 1) * 128],
                                        xn4[:, kp, :, :],
                                        start=(kp == 0),
                                        stop=(kp == KT // 2 - 1),
                                        perf_mode=PM.DoubleRow)
                                nc.scalar.activation(
                                    hj[:, jj, :], f1[:, jj, :], AF.Gelu,
                                    bias=b1_sb[:, j:j + 1], scale=1.0 / 16.0)
                            for m in range(KT):
                                nc.tensor.matmul(
                                    f2[m][:],
                                    w2_sb[:, jp, :, m * 128:(m + 1) * 128],
                                    hj[:, :, :], start=(jp == 0),
                                    stop=(jp == NJP - 1),
                                    perf_mode=PM.DoubleRow)
                        for m in range(KT):
                            fo = work.tile([128, TPC], F32, tag="fo")
                            nc.vector.scalar_tensor_tensor(
                                fo[:], f2[m][:], 1.0 / 16.0,
                                x_sb[:, m, :], ALU.mult, ALU.add)
                            nc.vector.tensor_scalar(
                                fo[:], fo[:], b2_sb[:, m:m + 1], None,
                                ALU.add, ALU.bypass)
                            nc.sync.dma_start(
                                out_t[m * 128:(m + 1) * 128, :], fo[:])

    _split_multi_waits(nc)
    return nc


_NC_CACHE = None


def _get_nc():
    global _NC_CACHE
    if _NC_CACHE is None:
        _NC_CACHE = build_nc()
    return _NC_CACHE


def _make_tabcs():
    # rows indexed by combined position py*64+px:
    # [cos_y(16)|cos_x(16)|sin_y(16)|sin_x(16)] in bf16
    j = np.arange(16)
    f = 1.0 / (ROPE_THETA ** (2.0 * j / 32.0))
    v = np.arange(64)
    cos = np.cos(v[:, None] * f[None, :])
    sin = np.sin(v[:, None] * f[None, :])
    py = np.repeat(np.arange(64), 64)
    px = np.tile(np.arange(64), 64)
    tab = np.concatenate(
        [cos[py], cos[px], sin[py], sin[px]], axis=1)
    return tab.astype(ml_dtypes.bfloat16)


def kernel(**inputs):
    from concourse.bass_utils import run_bass_kernel_spmd

    np32 = lambda x: np.asarray(x, dtype=np.float32)
    npbf = lambda x: np.asarray(np.asarray(x, dtype=np.float32),
                                dtype=ml_dtypes.bfloat16)
    query = np32(inputs["query"])
    kvin = np32(inputs["kv"])
    pos_q = np.asarray(inputs["pos_q"]).astype(np.int64)
    pos_kv = np.asarray(inputs["pos_kv"]).astype(np.int64)

    lnqw, lnqb = np32(inputs["ln_q_w"]), np32(inputs["ln_q_b"])
    lnkw, lnkb = np32(inputs["ln_kv_w"]), np32(inputs["ln_kv_b"])
    lnmw, lnmb = np32(inputs["ln_mlp_w"]), np32(inputs["ln_mlp_b"])
    wq = np32(inputs["wq"]) * lnqw[:, None]
    wk = np32(inputs["wk"]) * lnkw[:, None]
    wv = np32(inputs["wv"]) * lnkw[:, None]
    bq = np32(inputs["bq"]) + lnqb @ np32(inputs["wq"])
    bk = np32(inputs["bk"]) + lnkb @ np32(inputs["wk"])
    bv = np32(inputs["bv"]) + lnkb @ np32(inputs["wv"])
    wo = np32(inputs["wo"])
    w1 = np32(inputs["w1"]) * lnmw[:, None]
    b1 = np32(inputs["b1"]) + lnmb @ np32(inputs["w1"])
    w2 = npbf(inputs["w2"])
    b2 = np32(inputs["b2"])
    w2_f8 = np.ascontiguousarray(
        np.clip(np32(inputs["w2"]) * 16.0, -240, 240)
        .reshape(12, 2, 128, DIM).transpose(2, 0, 1, 3)
    ).astype(ml_dtypes.float8_e4m3)
    w1_f8 = np.ascontiguousarray(
        np.clip(w1 * 16.0, -240, 240).reshape(3, 2, 128, MLP_H)
        .transpose(2, 0, 1, 3)).astype(ml_dtypes.float8_e4m3)
    tabcs = _make_tabcs()

    def posc(pos_b):  # [P, 2] -> [128, NT] combined per-tile-partition
        pc = (pos_b[:, 0] * 64 + pos_b[:, 1]).astype(np.int32)
        return np.ascontiguousarray(
            pc.reshape(NT, 128).T)

    in_maps = []
    for c in range(N_CORES):
        b, s = c // G, c % G
        hs = slice(HPC * DH * s, HPC * DH * (s + 1))
        ts = slice(TPC * s, TPC * (s + 1))
        wo_s = wo[hs, :]
        wo2d = np.concatenate([wo_s[128:192], wo_s[128:192]], axis=0)
        in_maps.append({
            "query": query[b],
            "kv": kvin[b],
            "q_res_t": np.ascontiguousarray(query[b, ts, :].T),
            "poscq": posc(pos_q[b]),
            "posckv": posc(pos_kv[b]),
            "tabcs": tabcs,
            "wq_s": np.ascontiguousarray(wq[:, hs]).astype(ml_dtypes.bfloat16),
            "wkv_s": np.ascontiguousarray(np.concatenate(
                [wk[:, hs], wv[:, hs]], axis=1)).astype(ml_dtypes.bfloat16),
            "bqkv_s": np.concatenate([bq[hs], bk[hs], bv[hs]]),
            "wo01_d": np.ascontiguousarray(
                wo_s[0:128]).astype(ml_dtypes.bfloat16),
            "wo2d_d": np.ascontiguousarray(wo2d).astype(ml_dtypes.bfloat16),
            "bo4": np32(inputs["bo"]) / G,
            "w1": w1_f8,
            "b1": b1,
            "w2": w2_f8,
            "b2": b2,
        })

    nc = _get_nc()
    res = run_bass_kernel_spmd(nc, in_maps, core_ids=list(range(N_CORES)))

    out = np.empty((B, P, DIM), np.float32)
    for c in range(N_CORES):
        b, s = c // G, c % G
        out[b, TPC * s:TPC * (s + 1), :] = res.results[c]["out_t"].T
    return out
